# revision 22
# baseline (speedup 1.0000x reference)
"""GalaxyTileDecoder on 8 Trainium2 NeuronCores.

The reference pipeline (linear decode -> zero-pad -> gate -> bilinear
grid_sample -> sum over M=2 sources) collapses algebraically: the sample
grid is a pure per-source translation, sampling the padded 53x53 image at
(y, x) = (i + 2.5 - 4*locs[...,0], j + 2.5 - 4*locs[...,1]).  Folding the
integer shift (one-hot over 6 positions per axis), the bilinear weights,
the decoder bias, the galaxy_bool gate, and the M-source sum into an
expanded feature dimension turns the whole forward into one matmul:

    out[p, :] = z_exp[p, :] @ W_exp          (K = 6*6*9 = 324)

with W_exp[(a, b, f), (i, j)] = canvas9[f, a+i, b+j] the 6x6 shifted
52x52 windows of the 9 basis images (8 decoder rows + bias) in a 57x57
zero canvas, and z_exp the bool * z9[f] * wy[a] * wx[b] coefficients
summed over the M=2 sources.

Two refinements over the plain 3-chunk K=324 matmul:

1. Each source only touches two adjacent y-shift slots {m+2, m+3}, so
   with K ordered (a, b, f) its 108 K-rows live in exactly one of five
   overlapping row windows of W_exp (see TILES).  Bucketing ptiles by
   their two sources' (min m, max m) yields batches whose union of
   active K-rows fits two disjoint 108-row windows, so nearly every
   batch streams the 2704 output columns only twice (22 passes total
   instead of the naive 30).  The batch structure is computed from the
   data at runtime and the program is compiled (and cached) per
   structure.

2. The output is written to DRAM in bf16 (~0.2% rounding, gate is 2e-2)
   and upcast on the host, halving the dominant write traffic.

Data parallel over ptiles: 10 batches of 128 per core, no collectives.
"""

import math
import os

import numpy as np

P_TOTAL = 10000
M = 2
N_CORES = 8
F = 9                            # 8 decoder features + bias
A = 6                            # y-shift positions
B = 6                            # x-shift positions
K = A * B * F                    # 324 expanded features
CH_ROWS = 2 * B * F              # 108 rows per a-pair chunk
OUT_HW = 52
COLS = OUT_HW * OUT_HW           # 2704
CANVAS = 57
BATCH = 128
UNIT = BATCH * N_CORES           # rows consumed by one batch slot across cores

_DT_NAME = os.environ.get("BASS_GAL_DT", "bf16")

_cache = {}

# output columns split into 1-PSUM-bank segments (<=512 f32); the final
# 144-wide remainder rides with the others
SEG6 = [(0, 512), (512, 1024), (1024, 1536), (1536, 2048),
        (2048, 2560), (2560, COLS)]

# W_exp row windows (108 rows each).  0/1/2 are the aligned a-pair chunks;
# 3/4 are the offset windows for sources with odd m (m=-1 -> slots {1,2},
# m=1 -> slots {3,4}).  Each source's two active a-slots live entirely in
# exactly one window: m=-2 -> 0, m=-1 -> 3, m=0 -> 1, m=1 -> 4, m=2 -> 2.
TILES = {0: (0, 108), 1: (108, 216), 2: (216, 324),
         3: (54, 162), 4: (162, 270)}


def _build_program(dt_name, struct):
    """struct: tuple of chunk-tuples, one per batch, e.g. ((0,1),(1,2),(0,1,2),...)."""
    import concourse.bass as bass  # noqa: F401  (registers engines)
    import concourse.tile as tile
    from concourse import bacc, mybir

    dt_map = {
        "bf16": mybir.dt.bfloat16,
        "f32": mybir.dt.float32,
    }
    DT = dt_map[dt_name]

    n_batches = len(struct)
    n_pass = sum(len(ch) for ch, _ in struct)
    # flat pass -> zt column offset
    pass_col = []
    off = 0
    for ch_list, _ in struct:
        cols = []
        for _ in ch_list:
            cols.append(off)
            off += BATCH
        pass_col.append(cols)

    nc = bacc.Bacc(trn_type="TRN2")
    zt = nc.dram_tensor("zt", [CH_ROWS, n_pass * BATCH], DT, kind="ExternalInput")
    wx = nc.dram_tensor("wx", [K, COLS], DT, kind="ExternalInput")
    out = nc.dram_tensor("out", [n_batches * BATCH, COLS], mybir.dt.bfloat16,
                         kind="ExternalOutput")

    # chunk order by first use; z split so the first batches' coefficients land early
    chunk_order = []
    for ch_list, _ in struct:
        for ch in ch_list:
            if ch not in chunk_order:
                chunk_order.append(ch)
    z_split = pass_col[min(2, n_batches - 1)][-1] + BATCH  # cols for batches 0..2

    with tile.TileContext(nc) as tc:
        with (
            tc.tile_pool(name="w", bufs=1) as wpool,
            tc.tile_pool(name="o", bufs=4) as opool,
            tc.tile_pool(name="ps", bufs=8, space="PSUM") as pspool,
        ):
            # PE warmup: dummy matmuls spanning the input-load phase.  Must
            # be >3.4us of SUSTAINED PE busy to trip the HAM clock-gate to
            # 2.4 GHz, and long enough to bridge until input data lands (a
            # >3.4us PE idle re-throttles to 1.2 GHz).
            warm = wpool.tile([128, 128], mybir.dt.bfloat16, tag="warm")
            nc.vector.memset(warm[:], 0.0)
            wps = pspool.tile([128, 512], mybir.dt.float32, tag="ps",
                              name="warmps")
            for _ in range(42):
                nc.tensor.matmul(wps[:, 0:128], warm[:, 0:128], warm[:, 0:128],
                                 start=True, stop=True)

            # inputs in first-use order (each trigger costs ~0.7us of
            # HWDGE descriptor-gen, so few big DMAs).  The second W tile
            # rides the scalar ring so it transfers concurrently with the
            # first + z0 on the sync ring, pulling batch 0's start earlier;
            # outputs only need the scalar ring from ~15us on.
            w_tiles = {ch: wpool.tile([CH_ROWS, COLS], DT, tag=f"w{ch}",
                                      name=f"w{ch}")
                       for ch in chunk_order}
            z0 = wpool.tile([CH_ROWS, z_split], DT, tag="z0")
            z1 = None
            if n_pass * BATCH > z_split:
                z1 = wpool.tile([CH_ROWS, n_pass * BATCH - z_split], DT, tag="z1")

            def load_w(engine, ch):
                r0, r1 = TILES[ch]
                engine.dma_start(w_tiles[ch][:], wx[r0:r1, :])

            load_w(nc.sync, chunk_order[0])
            nc.sync.dma_start(z0[:], zt[:, 0:z_split])
            if len(chunk_order) > 1:
                load_w(nc.scalar, chunk_order[1])
            if z1 is not None:
                nc.sync.dma_start(z1[:], zt[:, z_split:])
            for ch in chunk_order[2:]:
                load_w(nc.sync, ch)

            def z_slice(col):
                if col < z_split:
                    return z0[:, col:col + BATCH]
                return z1[:, col - z_split:col - z_split + BATCH]

            for bi, (ch_list, rb) in enumerate(struct):
                last = bi == len(struct) - 1
                osb = opool.tile([128, COLS], mybir.dt.bfloat16, tag="osb")
                for si, (s0, s1) in enumerate(SEG6):
                    sw = s1 - s0
                    ps = pspool.tile([128, 512], mybir.dt.float32, tag="ps")
                    for ji, ch in enumerate(ch_list):
                        zsl = z_slice(pass_col[bi][ji])
                        nc.tensor.matmul(
                            ps[0:128, 0:sw],
                            zsl,
                            w_tiles[ch][:, s0:s1],
                            start=(ji == 0),
                            stop=(ji == len(ch_list) - 1),
                        )
                    # evacuate PSUM per segment, alternating DVE/ACT (~1x
                    # PSUM-read rate each); per-seg slots recycle fast enough
                    # that the PE never waits on a CAST
                    if si % 2 == 0:
                        nc.scalar.copy(osb[:, s0:s1], ps[0:128, 0:sw])
                    else:
                        nc.vector.tensor_copy(osb[:, s0:s1], ps[0:128, 0:sw])
                    if last and si % 2 == 1:
                        # tail: ship as soon as a 1024-wide slice lands
                        nc.scalar.dma_start(
                            out[bi * BATCH:bi * BATCH + rb, s0 - 512:s1],
                            osb[0:rb, s0 - 512:s1])
                if not last:
                    # outputs ride the scalar HWDGE ring so they never queue
                    # behind input transfers on the sync ring (FIFO per ring);
                    # only the rb real rows of this batch are written
                    nc.scalar.dma_start(out[bi * BATCH:bi * BATCH + rb, :],
                                        osb[0:rb, :])
    nc.compile()
    return nc


def _get_program(dt_name, struct):
    key = (dt_name, struct)
    if key not in _cache:
        _cache[key] = _build_program(dt_name, struct)
    return _cache[key]


def _plan_batches(mlo, mhi):
    """mlo/mhi: (P,) min/max source y-shift index (m in -2..2) per ptile.
    Returns (struct, batch_ids): struct is a tuple of W-tile-id tuples (see
    TILES), batch_ids is (n_batches, N_CORES, BATCH) int32 ptile ids (-1 pad).

    Every batch's tile ranges are pairwise disjoint and jointly cover the
    active K-rows of all its ptiles, so most batches need only 2 column
    passes.  Leftovers that fit no 2-tile bucket go to 3-pass overflow."""
    P = mlo.shape[0]
    ids = np.arange(P)
    is_flex = (mlo == 0) & (mhi == 0)
    groups = {
        "A": ids[(mhi <= 0) & ~is_flex],     # tiles (0, 1)
        "B": ids[(mlo >= 0) & ~is_flex],     # tiles (1, 2)
        "C1": ids[(mlo == -2) & (mhi == 1)],  # (0, 4)
        "C2": ids[(mlo == -2) & (mhi == 2)],  # (0, 2)
        "C3": ids[(mlo == -1) & (mhi == 1)],  # (3, 4)
        "C4": ids[(mlo == -1) & (mhi == 2)],  # (3, 2)
    }
    flex = ids[is_flex]   # both sources m=0 (slots {2,3}): fits A, B, or C3

    rows = {g: [v] for g, v in groups.items()}
    counts = {g: len(v) for g, v in groups.items()}

    def top_up(g):
        nonlocal flex
        n_full = counts[g] // UNIT
        left = counts[g] - n_full * UNIT
        if left and len(flex) >= UNIT - left:
            rows[g].append(flex[:UNIT - left])
            flex = flex[UNIT - left:]
            counts[g] += UNIT - left
            n_full += 1
        return n_full

    nA = top_up("A")
    nB = top_up("B")
    # remaining flex rides with C3 (slots {2,3} inside windows {1,2}+{3,4})
    rows["C3"].append(flex)
    counts["C3"] += len(flex)

    tile_sets = {"A": (0, 1), "B": (1, 2), "C1": (0, 4), "C2": (0, 2),
                 "C3": (3, 4), "C4": (3, 2)}

    def make_units(tile_set, cat):
        """Deal rows slot-major so per-core pads sit at the tail, letting the
        output DMA skip them: unit i writes only ceil(n_real/8) rows/core."""
        units = []
        for u0 in range(0, max(len(cat), 1), UNIT):
            part = cat[u0:u0 + UNIT]
            grid = np.full((N_CORES, BATCH), -1, np.int64)
            grid.T.flat[:len(part)] = part
            units.append((tile_set, grid, math.ceil(len(part) / N_CORES)))
        return units

    units = []
    over = []   # rows that fit no full unit of their group -> overflow
    for g, n_units in (("A", nA), ("B", nB)):
        cat = np.concatenate(rows[g])
        if n_units:
            units += make_units(tile_sets[g], cat[:n_units * UNIT])
        over.append(cat[n_units * UNIT:])
    c_units = []
    for g in ("C1", "C2", "C3", "C4"):
        cat = np.concatenate(rows[g])
        if len(cat):
            c_units += make_units(tile_sets[g], cat)
    over = np.concatenate(over)
    if len(over):
        c_units += make_units((0, 1, 2), over)
    # smallest batch last -> shortest final output transfer on the tail
    c_units.sort(key=lambda u: -u[2])
    units += c_units

    struct = tuple((ts, r) for ts, _, r in units)
    batch_ids = np.stack([g for _, g, _ in units]).astype(np.int32)
    return struct, batch_ids


def _host_expand(locs, galaxy_params, galaxy_bool, W_dec, b_dec, np_dtype):
    """Coefficients z_sum (P, K), chunk-need bitmask (P,), W_exp (K, COLS)."""
    locs = np.asarray(locs, np.float32).reshape(-1, 2)
    params = np.asarray(galaxy_params, np.float32).reshape(-1, 8)
    gbool = np.asarray(galaxy_bool, np.float32).reshape(-1, 1)
    W = np.asarray(W_dec, np.float32)
    b = np.asarray(b_dec, np.float32)
    N = locs.shape[0]
    P = N // M

    sy = 2.5 - 4.0 * locs[:, 0]
    sx = 2.5 - 4.0 * locs[:, 1]
    m = np.clip(np.floor(sy), -2, 2)
    k = np.clip(np.floor(sx), -2, 2)
    fy = (sy - m).astype(np.float32)
    fx = (sx - k).astype(np.float32)
    m = m.astype(np.int64)
    k = k.astype(np.int64)
    ar = np.arange(N)
    cy = np.zeros((N, A), np.float32)
    cx = np.zeros((N, B), np.float32)
    cy[ar, m + 2] = 1.0 - fy
    cy[ar, m + 3] = fy
    cx[ar, k + 2] = 1.0 - fx
    cx[ar, k + 3] = fx

    z9 = np.concatenate([params, np.ones((N, 1), np.float32)], axis=1) * gbool
    z_exp = (cy[:, :, None, None] * cx[:, None, :, None] * z9[:, None, None, :])
    z_sum = z_exp.reshape(P, M, K).sum(axis=1).astype(np_dtype)

    mp = m.reshape(P, M)
    mlo = mp.min(axis=1)
    mhi = mp.max(axis=1)

    canvas9 = np.zeros((F, CANVAS, CANVAS), np.float32)
    canvas9[:8, 3:54, 3:54] = W.reshape(8, 51, 51)
    canvas9[8, 3:54, 3:54] = b.reshape(51, 51)
    sw = np.lib.stride_tricks.sliding_window_view(canvas9, (OUT_HW, OUT_HW), axis=(1, 2))
    Wexp = np.ascontiguousarray(
        sw.transpose(1, 2, 0, 3, 4).reshape(K, COLS), dtype=np_dtype)
    return z_sum, mlo, mhi, Wexp


def kernel(locs, galaxy_params, galaxy_bool, W_dec, b_dec, _trace=False):
    import ml_dtypes
    from concourse.bass_utils import run_bass_kernel_spmd

    np_dtype = {
        "bf16": ml_dtypes.bfloat16,
        "f32": np.float32,
    }[_DT_NAME]

    z_sum, mlo, mhi, Wexp = _host_expand(
        locs, galaxy_params, galaxy_bool, W_dec, b_dec, np_dtype)
    struct, batch_ids = _plan_batches(mlo, mhi)
    n_batches = len(struct)
    n_pass = sum(len(ch) for ch, _ in struct)

    # per-core zt: [108, n_pass*128], one 128-col block per (batch, tile) pass
    z_pad = np.concatenate([z_sum, np.zeros((1, K), np_dtype)], axis=0)  # -1 -> 0
    zt = np.empty((N_CORES, CH_ROWS, n_pass * BATCH), np_dtype)
    for c in range(N_CORES):
        off = 0
        for bi, (ch_list, _) in enumerate(struct):
            rows = batch_ids[bi, c]
            zb = z_pad[rows]                       # (128, K)
            for ch in ch_list:
                r0, r1 = TILES[ch]
                zt[c, :, off:off + BATCH] = zb[:, r0:r1].T
                off += BATCH

    nc = _get_program(_DT_NAME, struct)
    in_maps = [{"zt": zt[c], "wx": Wexp} for c in range(N_CORES)]
    kwargs = {}
    if _trace:
        kwargs["trace"] = True
    res = run_bass_kernel_spmd(nc, in_maps, core_ids=list(range(N_CORES)), **kwargs)

    P = z_sum.shape[0]
    full = np.empty((P + 1, COLS), np.float32)
    safe_ids = np.where(batch_ids < 0, P, batch_ids)   # (n_batches, N_CORES, 128)
    for c in range(N_CORES):
        co = np.asarray(res.results[c]["out"]).astype(np.float32)
        full[safe_ids[:, c, :].reshape(-1)] = co.reshape(n_batches * BATCH, COLS)
    out = full[:P].reshape(P, 1, OUT_HW, OUT_HW)
    if _trace:
        kernel._last_result = res
    return out, out


# revision 24
# speedup vs baseline: 1.4281x; 1.4281x over previous
"""GalaxyTileDecoder on 8 Trainium2 NeuronCores.

The reference pipeline (linear decode -> zero-pad -> gate -> bilinear
grid_sample -> sum over M=2 sources) collapses algebraically: the sample
grid is a pure per-source translation, sampling the padded 53x53 image at
(y, x) = (i + 2.5 - 4*locs[...,0], j + 2.5 - 4*locs[...,1]).  Folding the
integer shift (one-hot over 6 positions per axis), the bilinear weights,
the decoder bias, the galaxy_bool gate, and the M-source sum into an
expanded feature dimension turns the whole forward into one matmul:

    out[p, :] = z_exp[p, :] @ W_exp          (K = 6*6*9 = 324)

with W_exp[(a, b, f), (i, j)] = canvas9[f, a+i, b+j] the 6x6 shifted
52x52 windows of the 9 basis images (8 decoder rows + bias) in a 57x57
zero canvas, and z_exp the bool * z9[f] * wy[a] * wx[b] coefficients
summed over the M=2 sources.

Two refinements over the plain 3-chunk K=324 matmul:

1. Each source only touches two adjacent y-shift slots {m+2, m+3}, so
   with K ordered (a, b, f) its 108 K-rows live in exactly one of five
   overlapping row windows of W_exp (see TILES).  Bucketing ptiles by
   their two sources' (min m, max m) yields batches whose union of
   active K-rows fits two disjoint 108-row windows, so nearly every
   batch streams the 2704 output columns only twice (22 passes total
   instead of the naive 30).  The batch structure is computed from the
   data at runtime and the program is compiled (and cached) per
   structure.

2. The output is written to DRAM in bf16 (~0.2% rounding, gate is 2e-2)
   and upcast on the host, halving the dominant write traffic.

Data parallel over ptiles: 10 batches of 128 per core, no collectives.
"""

import math
import os

import numpy as np

P_TOTAL = 10000
M = 2
N_CORES = 8
F = 9                            # 8 decoder features + bias
A = 6                            # y-shift positions
B = 6                            # x-shift positions
K = A * B * F                    # 324 expanded features
CH_ROWS = 2 * B * F              # 108 rows per a-pair chunk
OUT_HW = 52
COLS = OUT_HW * OUT_HW           # 2704
CANVAS = 57
BATCH = 128
UNIT = BATCH * N_CORES           # rows consumed by one batch slot across cores

_DT_NAME = os.environ.get("BASS_GAL_DT", "bf16")

_cache = {}

# output columns split into 1-PSUM-bank segments (<=512 f32); the final
# 144-wide remainder rides with the others
SEG6 = [(0, 512), (512, 1024), (1024, 1536), (1536, 2048),
        (2048, 2560), (2560, COLS)]

# W_exp row windows (108 rows each).  0/1/2 are the aligned a-pair chunks;
# 3/4 are the offset windows for sources with odd m (m=-1 -> slots {1,2},
# m=1 -> slots {3,4}).  Each source's two active a-slots live entirely in
# exactly one window: m=-2 -> 0, m=-1 -> 3, m=0 -> 1, m=1 -> 4, m=2 -> 2.
TILES = {0: (0, 108), 1: (108, 216), 2: (216, 324),
         3: (54, 162), 4: (162, 270)}


def _build_program(dt_name, struct):
    """struct: tuple of chunk-tuples, one per batch, e.g. ((0,1),(1,2),(0,1,2),...)."""
    import concourse.bass as bass  # noqa: F401  (registers engines)
    import concourse.tile as tile
    from concourse import bacc, mybir

    dt_map = {
        "bf16": mybir.dt.bfloat16,
        "f32": mybir.dt.float32,
    }
    DT = dt_map[dt_name]

    n_batches = len(struct)
    n_pass = sum(len(ch) for ch in struct)
    # flat pass -> zt column offset
    pass_col = []
    off = 0
    for ch_list in struct:
        cols = []
        for _ in ch_list:
            cols.append(off)
            off += BATCH
        pass_col.append(cols)

    nc = bacc.Bacc(trn_type="TRN2")
    zt = nc.dram_tensor("zt", [CH_ROWS, n_pass * BATCH], DT, kind="ExternalInput")
    wx = nc.dram_tensor("wx", [K, COLS], DT, kind="ExternalInput")
    out = nc.dram_tensor("out", [n_batches * BATCH, COLS], mybir.dt.bfloat16,
                         kind="ExternalOutput")

    # chunk order by first use; z split so the first batches' coefficients land early
    chunk_order = []
    for ch_list in struct:
        for ch in ch_list:
            if ch not in chunk_order:
                chunk_order.append(ch)
    z_split = pass_col[min(2, n_batches - 1)][-1] + BATCH  # cols for batches 0..2

    with tile.TileContext(nc) as tc:
        with (
            tc.tile_pool(name="w", bufs=1) as wpool,
            tc.tile_pool(name="o", bufs=4) as opool,
            tc.tile_pool(name="ps", bufs=8, space="PSUM") as pspool,
        ):
            # PE warmup: dummy matmuls spanning the input-load phase.  Must
            # be >3.4us of SUSTAINED PE busy to trip the HAM clock-gate to
            # 2.4 GHz, and long enough to bridge until input data lands (a
            # >3.4us PE idle re-throttles to 1.2 GHz).
            warm = wpool.tile([128, 128], mybir.dt.bfloat16, tag="warm")
            nc.vector.memset(warm[:], 0.0)
            wps = pspool.tile([128, 512], mybir.dt.float32, tag="ps",
                              name="warmps")
            for _ in range(42):
                nc.tensor.matmul(wps[:, 0:128], warm[:, 0:128], warm[:, 0:128],
                                 start=True, stop=True)

            # inputs in first-use order (each trigger costs ~0.7us of
            # HWDGE descriptor-gen, so few big DMAs).  The second W tile
            # rides the scalar ring so it transfers concurrently with the
            # first + z0 on the sync ring, pulling batch 0's start earlier;
            # outputs only need the scalar ring from ~15us on.
            w_tiles = {ch: wpool.tile([CH_ROWS, COLS], DT, tag=f"w{ch}",
                                      name=f"w{ch}")
                       for ch in chunk_order}
            z0 = wpool.tile([CH_ROWS, z_split], DT, tag="z0")
            z1 = None
            if n_pass * BATCH > z_split:
                z1 = wpool.tile([CH_ROWS, n_pass * BATCH - z_split], DT, tag="z1")

            def load_w(engine, ch):
                r0, r1 = TILES[ch]
                engine.dma_start(w_tiles[ch][:], wx[r0:r1, :])

            load_w(nc.sync, chunk_order[0])
            nc.sync.dma_start(z0[:], zt[:, 0:z_split])
            if len(chunk_order) > 1:
                load_w(nc.scalar, chunk_order[1])
            if z1 is not None:
                nc.sync.dma_start(z1[:], zt[:, z_split:])
            for ch in chunk_order[2:]:
                load_w(nc.sync, ch)

            def z_slice(col):
                if col < z_split:
                    return z0[:, col:col + BATCH]
                return z1[:, col - z_split:col - z_split + BATCH]

            for bi, ch_list in enumerate(struct):
                last = bi == len(struct) - 1
                osb = opool.tile([128, COLS], mybir.dt.bfloat16, tag="osb")
                for si, (s0, s1) in enumerate(SEG6):
                    sw = s1 - s0
                    ps = pspool.tile([128, 512], mybir.dt.float32, tag="ps")
                    for ji, ch in enumerate(ch_list):
                        zsl = z_slice(pass_col[bi][ji])
                        nc.tensor.matmul(
                            ps[0:128, 0:sw],
                            zsl,
                            w_tiles[ch][:, s0:s1],
                            start=(ji == 0),
                            stop=(ji == len(ch_list) - 1),
                        )
                    # evacuate PSUM per segment, alternating DVE/ACT (~1x
                    # PSUM-read rate each); per-seg slots recycle fast enough
                    # that the PE never waits on a CAST
                    if si % 2 == 0:
                        nc.scalar.copy(osb[:, s0:s1], ps[0:128, 0:sw])
                    else:
                        nc.vector.tensor_copy(osb[:, s0:s1], ps[0:128, 0:sw])
                    if last and si % 2 == 1:
                        # tail: ship as soon as a 1024-wide slice lands
                        nc.scalar.dma_start(
                            out[bi * BATCH:(bi + 1) * BATCH, s0 - 512:s1],
                            osb[:, s0 - 512:s1])
                if not last:
                    # outputs ride the scalar HWDGE ring so they never queue
                    # behind input transfers on the sync ring (FIFO per ring)
                    nc.scalar.dma_start(out[bi * BATCH:(bi + 1) * BATCH, :],
                                        osb[:])
    nc.compile()
    return nc


def _get_program(dt_name, struct):
    key = (dt_name, struct)
    if key not in _cache:
        _cache[key] = _build_program(dt_name, struct)
    return _cache[key]


def _plan_batches(mlo, mhi):
    """mlo/mhi: (P,) min/max source y-shift index (m in -2..2) per ptile.
    Returns (struct, batch_ids): struct is a tuple of W-tile-id tuples (see
    TILES), batch_ids is (n_batches, N_CORES, BATCH) int32 ptile ids (-1 pad).

    Every batch's tile ranges are pairwise disjoint and jointly cover the
    active K-rows of all its ptiles, so most batches need only 2 column
    passes.  Leftovers that fit no 2-tile bucket go to 3-pass overflow."""
    P = mlo.shape[0]
    ids = np.arange(P)
    is_flex = (mlo == 0) & (mhi == 0)
    groups = {
        "A": ids[(mhi <= 0) & ~is_flex],     # tiles (0, 1)
        "B": ids[(mlo >= 0) & ~is_flex],     # tiles (1, 2)
        "C1": ids[(mlo == -2) & (mhi == 1)],  # (0, 4)
        "C2": ids[(mlo == -2) & (mhi == 2)],  # (0, 2)
        "C3": ids[(mlo == -1) & (mhi == 1)],  # (3, 4)
        "C4": ids[(mlo == -1) & (mhi == 2)],  # (3, 2)
    }
    flex = ids[is_flex]   # both sources m=0 (slots {2,3}): fits A, B, or C3

    rows = {g: [v] for g, v in groups.items()}
    counts = {g: len(v) for g, v in groups.items()}

    def top_up(g):
        nonlocal flex
        n_full = counts[g] // UNIT
        left = counts[g] - n_full * UNIT
        if left and len(flex) >= UNIT - left:
            rows[g].append(flex[:UNIT - left])
            flex = flex[UNIT - left:]
            counts[g] += UNIT - left
            n_full += 1
        return n_full

    nA = top_up("A")
    nB = top_up("B")
    # remaining flex rides with C3 (slots {2,3} inside windows {1,2}+{3,4})
    rows["C3"].append(flex)
    counts["C3"] += len(flex)

    tile_sets = {"A": (0, 1), "B": (1, 2), "C1": (0, 4), "C2": (0, 2),
                 "C3": (3, 4), "C4": (3, 2)}

    # Batch count is pinned to ceil(P/UNIT): every extra batch costs a full
    # 128x2704 output write per core, which outweighs any pass savings.  The
    # total pad budget is therefore fixed; spend it turning near-full C
    # groups into pure 2-pass units (cheapest pads first), and send the rest
    # to universal 3-pass (0,1,2) overflow units.
    n_units_target = math.ceil(P / UNIT)
    pad_budget = n_units_target * UNIT - P

    units = []   # (tile_set, rows_array possibly short of UNIT)
    over = []
    for g, n_units in (("A", nA), ("B", nB)):
        cat = np.concatenate(rows[g])
        for u in range(n_units):
            units.append((tile_sets[g], cat[u * UNIT:(u + 1) * UNIT]))
        over.append(cat[n_units * UNIT:])

    rests = []
    for g in ("C1", "C2", "C3", "C4"):
        cat = np.concatenate(rows[g])
        n_full = len(cat) // UNIT
        for u in range(n_full):
            units.append((tile_sets[g], cat[u * UNIT:(u + 1) * UNIT]))
        if len(cat) > n_full * UNIT:
            rests.append((tile_sets[g], cat[n_full * UNIT:]))
    # cheapest-to-pad rests become pure 2-pass units while budget lasts
    rests.sort(key=lambda r: UNIT - len(r[1]))
    for ts, cat in rests:
        pad = UNIT - len(cat)
        if pad <= pad_budget:
            pad_budget -= pad
            units.append((ts, cat))
        else:
            over.append(cat)

    over = np.concatenate(over)
    for u0 in range(0, len(over), UNIT):
        units.append(((0, 1, 2), over[u0:u0 + UNIT]))

    struct = tuple(ts for ts, _ in units)
    batch_ids = np.full((len(units), UNIT), -1, np.int64)
    for i, (_, cat) in enumerate(units):
        batch_ids[i, :len(cat)] = cat
    batch_ids = batch_ids.reshape(len(units), N_CORES, BATCH).astype(np.int32)
    return struct, batch_ids


def _host_expand(locs, galaxy_params, galaxy_bool, W_dec, b_dec, np_dtype):
    """Coefficients z_sum (P, K), chunk-need bitmask (P,), W_exp (K, COLS)."""
    locs = np.asarray(locs, np.float32).reshape(-1, 2)
    params = np.asarray(galaxy_params, np.float32).reshape(-1, 8)
    gbool = np.asarray(galaxy_bool, np.float32).reshape(-1, 1)
    W = np.asarray(W_dec, np.float32)
    b = np.asarray(b_dec, np.float32)
    N = locs.shape[0]
    P = N // M

    sy = 2.5 - 4.0 * locs[:, 0]
    sx = 2.5 - 4.0 * locs[:, 1]
    m = np.clip(np.floor(sy), -2, 2)
    k = np.clip(np.floor(sx), -2, 2)
    fy = (sy - m).astype(np.float32)
    fx = (sx - k).astype(np.float32)
    m = m.astype(np.int64)
    k = k.astype(np.int64)
    ar = np.arange(N)
    cy = np.zeros((N, A), np.float32)
    cx = np.zeros((N, B), np.float32)
    cy[ar, m + 2] = 1.0 - fy
    cy[ar, m + 3] = fy
    cx[ar, k + 2] = 1.0 - fx
    cx[ar, k + 3] = fx

    z9 = np.concatenate([params, np.ones((N, 1), np.float32)], axis=1) * gbool
    z_exp = (cy[:, :, None, None] * cx[:, None, :, None] * z9[:, None, None, :])
    z_sum = z_exp.reshape(P, M, K).sum(axis=1).astype(np_dtype)

    mp = m.reshape(P, M)
    mlo = mp.min(axis=1)
    mhi = mp.max(axis=1)

    canvas9 = np.zeros((F, CANVAS, CANVAS), np.float32)
    canvas9[:8, 3:54, 3:54] = W.reshape(8, 51, 51)
    canvas9[8, 3:54, 3:54] = b.reshape(51, 51)
    sw = np.lib.stride_tricks.sliding_window_view(canvas9, (OUT_HW, OUT_HW), axis=(1, 2))
    Wexp = np.ascontiguousarray(
        sw.transpose(1, 2, 0, 3, 4).reshape(K, COLS), dtype=np_dtype)
    return z_sum, mlo, mhi, Wexp


def kernel(locs, galaxy_params, galaxy_bool, W_dec, b_dec, _trace=False):
    import ml_dtypes
    from concourse.bass_utils import run_bass_kernel_spmd

    np_dtype = {
        "bf16": ml_dtypes.bfloat16,
        "f32": np.float32,
    }[_DT_NAME]

    z_sum, mlo, mhi, Wexp = _host_expand(
        locs, galaxy_params, galaxy_bool, W_dec, b_dec, np_dtype)
    struct, batch_ids = _plan_batches(mlo, mhi)
    n_batches = len(struct)
    n_pass = sum(len(ch) for ch in struct)

    # per-core zt: [108, n_pass*128], one 128-col block per (batch, tile) pass
    z_pad = np.concatenate([z_sum, np.zeros((1, K), np_dtype)], axis=0)  # -1 -> 0
    zt = np.empty((N_CORES, CH_ROWS, n_pass * BATCH), np_dtype)
    for c in range(N_CORES):
        off = 0
        for bi, ch_list in enumerate(struct):
            rows = batch_ids[bi, c]
            zb = z_pad[rows]                       # (128, K)
            for ch in ch_list:
                r0, r1 = TILES[ch]
                zt[c, :, off:off + BATCH] = zb[:, r0:r1].T
                off += BATCH

    nc = _get_program(_DT_NAME, struct)
    in_maps = [{"zt": zt[c], "wx": Wexp} for c in range(N_CORES)]
    kwargs = {}
    if _trace:
        kwargs["trace"] = True
    res = run_bass_kernel_spmd(nc, in_maps, core_ids=list(range(N_CORES)), **kwargs)

    P = z_sum.shape[0]
    full = np.empty((P + 1, COLS), np.float32)
    safe_ids = np.where(batch_ids < 0, P, batch_ids)   # (n_batches, N_CORES, 128)
    for c in range(N_CORES):
        co = np.asarray(res.results[c]["out"]).astype(np.float32)
        full[safe_ids[:, c, :].reshape(-1)] = co.reshape(n_batches * BATCH, COLS)
    out = full[:P].reshape(P, 1, OUT_HW, OUT_HW)
    if _trace:
        kernel._last_result = res
    return out, out
